# revision 1
# baseline (speedup 1.0000x reference)
"""CapsuleLayer dynamic-routing kernel for 8 Trainium2 NeuronCores.

Math (reference):
    u_hat[b,n,j,d] = sum_i W[n,j,d,i] * x[b,j,i]
    b = 0; for r in 0..2:
        c = softmax_n(b); s[b,n,d] = sum_j c*u_hat; v = squash_d(s)
        if r < 2: b += sum_d v*u_hat
    return v  [B, N, D]

Sharding: J (input capsules, 2048) split 8 ways -> Jc=256 per core.
Softmax over n is local; only s needs a 256 KiB AllReduce per iteration.

Per-core dataflow, one sweep over W per routing iteration (u_hat is
recomputed from SBUF-streamed W each iteration; never materialized):
  - j processed in groups of 4: 4 PE sub-matmuls via column tiling
    (tile_position=(0,32r)) produce u_hat group tile
    [128=(4j x 32b), (n,d)] in PSUM.
  - r0: softmax(0) is uniform, so u_hat is accumulated over all j
    directly in PSUM; s0 = (1/N) * strip-sum. No vector work at all.
  - r>=1: logits[p=(j,b), (g,n)] += sum_d v_{r-1}*u_hat  (DVE mult +
    segmented reduce over d); softmax over n is local to each
    (partition, group) -> c; tmp2 = c (x) u_hat on GpSimd.
  - s accumulated over j by a PE matmul with a stacked-identity lhsT
    (sums the 4 j-strips per b), accumulating across groups in PSUM.
    s-matmuls are emitted one group late so they don't block the next
    group's u_hat matmuls in the in-order PE queue.
  - AllReduce s across cores, squash redundantly on every core.
"""

import functools
import numpy as np

B, J, I = 32, 2048, 16
N, D = 64, 32
NCORES = 8
JC = J // NCORES          # 256 j per core
GRP = 4                   # j's per group (PE column strips)
NG = JC // GRP            # 64 groups
ND = N * D                # 2048
HALF = ND // 2            # 1024 free-dim half (PSUM sizing)
NH = N // 2               # 32 n per half
ROUTINGS = 3
EPS = 1e-7


@functools.lru_cache(maxsize=1)
def _build():
    import concourse.bass as bass
    import concourse.mybir as mybir
    import concourse.bacc as bacc
    import concourse.tile as tile

    f32 = mybir.dt.float32
    bf16 = mybir.dt.bfloat16
    MUL = mybir.AluOpType.mult
    ADD = mybir.AluOpType.add
    AX = mybir.AxisListType.X
    AF = mybir.ActivationFunctionType

    nc = bacc.Bacc("TRN2", target_bir_lowering=False, debug=False,
                   num_devices=NCORES)

    xt_d = nc.dram_tensor("xt", [I, JC * B], bf16, kind="ExternalInput")
    wt_d = nc.dram_tensor("wt", [I, JC, ND], bf16, kind="ExternalInput")
    ones_d = nc.dram_tensor("ones4", [GRP * B, B], bf16, kind="ExternalInput")
    v_d = nc.dram_tensor("v", [B, ND], f32, kind="ExternalOutput")

    with tile.TileContext(nc) as tc:
        with (
            tc.tile_pool(name="persist", bufs=1) as pp,
            tc.tile_pool(name="wstream", bufs=4) as wp,
            tc.tile_pool(name="work", bufs=4) as wk,
            tc.tile_pool(name="small", bufs=6) as sm,
            tc.tile_pool(name="ups", bufs=3, space="PSUM") as ups_pool,
            tc.tile_pool(name="sps", bufs=1, space="PSUM") as sps_pool,
            tc.tile_pool(name="dram", bufs=1, space="DRAM") as dr,
        ):
            xt = pp.tile([I, JC * B], bf16)
            nc.sync.dma_start(xt[:], xt_d[:])
            ones4 = pp.tile([GRP * B, B], bf16)
            nc.sync.dma_start(ones4[:], ones_d[:])

            logits = pp.tile([128, NG, N], bf16)
            v_rep = pp.tile([128, N, D], bf16)
            v_small = pp.tile([B, ND], bf16)
            s_sb = pp.tile([128, 512], f32)
            v_sb = pp.tile([B, ND], f32)

            cc_in = dr.tile([128, 512], f32)
            cc_out = dr.tile([128, 512], f32)

            def u_mms(u_ps, w_t, g, h, start, stop):
                """16 col-tiled matmuls for one (group, half); rr-outer so
                consecutive chunk matmuls share the stationary lhsT."""
                for rr in range(GRP):
                    j = g * GRP + rr
                    for cch in range(2):
                        nc.tensor.matmul(
                            u_ps[32 * rr:32 * rr + 32,
                                 cch * 512:(cch + 1) * 512],
                            xt[:, j * B:(j + 1) * B],
                            w_t[:, rr, h * HALF + cch * 512:
                                h * HALF + (cch + 1) * 512],
                            start=start, stop=stop,
                            tile_position=(0, 32 * rr),
                            skip_group_check=True,
                        )

            for r in range(ROUTINGS):
                s_ps = sps_pool.tile([128, 512], f32)

                if r == 0:
                    # -- r0: c is uniform; accumulate u_hat over j in PSUM --
                    acc = [ups_pool.tile([128, HALF], f32, name=f"acc{_h}", tag="u_ps") for _h in range(2)]
                    for g in range(NG):
                        w_t = wp.tile([I, GRP, ND], bf16)
                        nc.sync.dma_start(
                            w_t[:], wt_d[:, g * GRP:(g + 1) * GRP, :])
                        for h in range(2):
                            u_mms(acc[h], w_t, g, h,
                                  start=(g == 0), stop=(g == NG - 1))
                    # evac to bf16 SBUF, then strip-sum via ones4 matmul
                    for h in range(2):
                        a_sb = wk.tile([128, HALF], bf16)
                        nc.scalar.activation(a_sb[:], acc[h][:], AF.Copy)
                        for cch in range(2):
                            q = 2 * h + cch
                            nc.tensor.matmul(
                                s_ps[32 * q:32 * q + 32, :],
                                ones4[:],
                                a_sb[:, cch * 512:(cch + 1) * 512],
                                start=True, stop=True,
                                tile_position=(0, 32 * q),
                                skip_group_check=True,
                            )
                else:
                    # -- r>=1: fused logits update + local softmax + s --
                    pending_smm = []
                    for g in range(NG):
                        w_t = wp.tile([I, GRP, ND], bf16)
                        nc.sync.dma_start(
                            w_t[:], wt_d[:, g * GRP:(g + 1) * GRP, :])

                        c_t = sm.tile([128, N], bf16)
                        zrec = sm.tile([128, 1], f32)

                        u_sb_halves = []
                        for h in range(2):
                            u_ps = ups_pool.tile([128, HALF], f32)
                            u_mms(u_ps, w_t, g, h, start=True, stop=True)
                            # emit previous group's s-matmuls behind this
                            # group's u-matmuls in the PE stream
                            if pending_smm:
                                pending_smm.pop(0)()

                            u_sb = wk.tile([128, NH, D], bf16)
                            nc.scalar.activation(u_sb[:], u_ps[:], AF.Copy)
                            tl = wk.tile([128, NH, D], bf16)
                            nc.vector.tensor_tensor(
                                tl[:], u_sb[:],
                                v_rep[:, h * NH:(h + 1) * NH, :], op=MUL)
                            with nc.allow_low_precision("bf16 routing logits"):
                                if r == 1:
                                    nc.vector.tensor_reduce(
                                        logits[:, g, h * NH:(h + 1) * NH],
                                        tl[:], axis=AX, op=ADD)
                                else:
                                    dtmp = sm.tile([128, NH], bf16)
                                    nc.vector.tensor_reduce(
                                        dtmp[:], tl[:], axis=AX, op=ADD)
                                    nc.vector.tensor_add(
                                        logits[:, g, h * NH:(h + 1) * NH],
                                        logits[:, g, h * NH:(h + 1) * NH],
                                        dtmp[:])
                            u_sb_halves.append(u_sb)

                        # local softmax over n for this group's 4 j's
                        e_t = sm.tile([128, N], f32)
                        nc.scalar.activation(e_t[:], logits[:, g, :], AF.Exp)
                        zsum = sm.tile([128, 1], f32)
                        nc.vector.tensor_reduce(zsum[:], e_t[:], axis=AX, op=ADD)
                        nc.vector.reciprocal(zrec[:], zsum[:])
                        nc.vector.tensor_scalar_mul(c_t[:], e_t[:], zrec[:])

                        # tmp2 = c (x) u_hat on GpSimd (c broadcast over d)
                        t2s = []
                        for h in range(2):
                            t2 = wk.tile([128, NH, D], bf16, name="t2", tag="t2")
                            eng = nc.vector if h == 0 else nc.gpsimd
                            eng.tensor_tensor(
                                t2[:], u_sb_halves[h][:],
                                c_t[:, h * NH:(h + 1) * NH, None]
                                .broadcast_to([128, NH, D]),
                                op=MUL)
                            t2s.append(t2)

                        def make_smm(t2s=t2s, g=g):
                            def emit():
                                for h in range(2):
                                    t2f = t2s[h][:].rearrange("p a b -> p (a b)")
                                    for cch in range(2):
                                        q = 2 * h + cch
                                        nc.tensor.matmul(
                                            s_ps[32 * q:32 * q + 32, :],
                                            ones4[:],
                                            t2f[:, cch * 512:(cch + 1) * 512],
                                            start=(g == 0), stop=(g == NG - 1),
                                            tile_position=(0, 32 * q),
                                            skip_group_check=True,
                                        )
                            return emit
                        pending_smm.append(make_smm())
                    while pending_smm:
                        pending_smm.pop(0)()

                # ---- end of sweep: AllReduce s, squash, update v ----
                # everything below stays in the (quarter, b)-strip layout:
                # partition 32q+b holds n in [16q,16q+16), all of d.
                s_evac = sm.tile([128, 512], f32)
                if r == 0:
                    nc.vector.tensor_scalar_mul(s_evac[:], s_ps[:], 1.0 / N)
                else:
                    nc.vector.tensor_copy(s_evac[:], s_ps[:])
                nc.sync.dma_start(cc_in[:], s_evac[:])
                nc.gpsimd.collective_compute(
                    "AllReduce", ADD,
                    replica_groups=[list(range(NCORES))],
                    ins=[cc_in[:].opt()], outs=[cc_out[:].opt()],
                )
                nc.sync.dma_start(s_sb[:], cc_out[:])

                sq = sm.tile([128, 16, D], f32)
                s3 = s_sb[:].rearrange("p (n d) -> p n d", d=D)
                nc.vector.tensor_tensor(sq[:], s3, s3, op=MUL)
                ns2 = sm.tile([128, 16], f32)
                nc.vector.tensor_reduce(ns2[:], sq[:], axis=AX, op=ADD)
                onep = sm.tile([128, 16], f32)
                nc.vector.tensor_scalar_add(onep[:], ns2[:], 1.0)
                rt = sm.tile([128, 16], f32)
                eps_t = sm.tile([128, 1], f32)
                nc.vector.memset(eps_t[:], EPS)
                nc.scalar.activation(rt[:], ns2[:], AF.Sqrt, bias=eps_t[:])
                den = sm.tile([128, 16], f32)
                nc.vector.tensor_tensor(den[:], onep[:], rt[:], op=MUL)
                dinv = sm.tile([128, 16], f32)
                nc.vector.reciprocal(dinv[:], den[:])
                scl = sm.tile([128, 16], f32)
                nc.vector.tensor_tensor(scl[:], ns2[:], dinv[:], op=MUL)
                v4 = sm.tile([128, 16, D], f32)
                nc.vector.tensor_tensor(
                    v4[:], s3,
                    scl[:, :, None].broadcast_to([128, 16, D]),
                    op=MUL)

                if r < ROUTINGS - 1:
                    v4b = sm.tile([128, 512], bf16)
                    nc.vector.tensor_copy(
                        v4b[:], v4[:].rearrange("p a b -> p (a b)"))
                    for q in range(4):
                        nc.sync.dma_start(
                            v_small[:, q * 512:(q + 1) * 512],
                            v4b[32 * q:32 * q + 32, :])
                    for rr in range(GRP):
                        nc.sync.dma_start(
                            v_rep[32 * rr:32 * rr + 32, :, :],
                            v_small[:].rearrange("b (n d) -> b n d", d=D))
                else:
                    for q in range(4):
                        nc.sync.dma_start(
                            v_sb[:, q * 512:(q + 1) * 512],
                            v4[32 * q:32 * q + 32, :])

            nc.sync.dma_start(v_d[:], v_sb[:])

    nc.compile()
    return nc


def kernel(x: np.ndarray, W: np.ndarray) -> np.ndarray:
    import ml_dtypes
    from concourse.bass_utils import run_bass_kernel_spmd

    nc = _build()

    bf = ml_dtypes.bfloat16
    xt = np.ascontiguousarray(x.transpose(2, 1, 0)).astype(bf)          # [I,J,B]
    wt = np.ascontiguousarray(W.transpose(3, 1, 0, 2).reshape(I, J, ND)).astype(bf)
    ones4 = np.tile(np.eye(B, dtype=np.float32), (GRP, 1)).astype(bf)

    in_maps = []
    for k in range(NCORES):
        jlo, jhi = k * JC, (k + 1) * JC
        in_maps.append({
            "xt": np.ascontiguousarray(xt[:, jlo:jhi, :]).reshape(I, JC * B),
            "wt": np.ascontiguousarray(wt[:, jlo:jhi, :]),
            "ones4": ones4,
        })

    res = run_bass_kernel_spmd(nc, in_maps, list(range(NCORES)))
    v = np.asarray(res.results[0]["v"], dtype=np.float32)
    return v.reshape(B, N, D)


if __name__ == "__main__":
    rng = np.random.default_rng(0)
    x = rng.normal(size=(B, J, I)).astype(np.float32)
    W = rng.normal(size=(N, J, D, I)).astype(np.float32) * 0.05
    v = kernel(x, W)
    print(v.shape, v.dtype, np.abs(v).max())



# revision 19
# speedup vs baseline: 1.2337x; 1.2337x over previous
"""CapsuleLayer dynamic-routing kernel for 8 Trainium2 NeuronCores (v4).

Math (reference):
    u_hat[b,n,j,d] = sum_i W[n,j,d,i] * x[b,j,i]
    b = 0; for r in 0..2:
        c = softmax_n(b); s[b,n,d] = sum_j c*u_hat; v = squash_d(s)
        if r < 2: b += sum_d v*u_hat
    return v  [B, N, D]

Key identities:
  - logits_r = <V_r, u_hat> over d with V_r = v_0 + ... + v_{r-1}
    (logits accumulate additively, u_hat constant) -> no per-j state.
  - r0: c uniform = 1/N, so s0 = (1/N) sum_{j,i} x W — computed with
    x-stationary K=128 matmuls (full PE-row packing), accumulating
    [32b, 2048] directly in PSUM.  No transposes.

Sharding: J (2048) split 8 ways -> Jc=256/core; s AllReduce per
iteration (256 KiB); squash redundant on every core.

Free layout everywhere: f = q*512 + d*16 + nn  (n = 16q + nn).
This keeps every DVE op innermost-step-1 bf16 (2x mode), makes each
s-matmul quarter a contiguous 512-col slice, and each strip of the
s PSUM [32q+b, (d,nn)] compacts to [32, (q,d,nn)] with contiguous DMAs.

Per r>=1 group (4 j): K=64 block-diag-x matmuls -> u_ps; scalar-ACT
evac; tl = u*Vrep (DVE 2x); 5 halving TT-adds fold d -> logits;
exp+Z fused on Scalar (accum_out); c = e/Z; t2 = c*u split DVE/GpSimd;
4 col-tiled ones4 matmuls accumulate s strips in PSUM.
"""

import functools
import numpy as np

B, J, I = 32, 2048, 16
N, D = 64, 32
NCORES = 8
JC = J // NCORES          # 256 j per core
GRP = 4                   # j's per group
NG = JC // GRP            # 64 groups
OCT = 8                   # j's per r0 octet (K=128 stationary)
NO = JC // OCT            # 32 octets
DN = D * N                # 2048
ROUTINGS = 3
EPS = 1e-7


@functools.lru_cache(maxsize=1)
def _build():
    import concourse.mybir as mybir
    import concourse.bacc as bacc
    import concourse.tile as tile

    f32 = mybir.dt.float32
    bf16 = mybir.dt.bfloat16
    MUL = mybir.AluOpType.mult
    ADD = mybir.AluOpType.add
    AF = mybir.ActivationFunctionType
    AX = mybir.AxisListType.X

    nc = bacc.Bacc("TRN2", target_bir_lowering=False, debug=False,
                   num_devices=NCORES)

    w1_d = nc.dram_tensor("w1", [NG, GRP * I, DN], bf16, kind="ExternalInput")
    w2_d = nc.dram_tensor("w2", [NO, OCT * I, DN], bf16, kind="ExternalInput")
    xr_d = nc.dram_tensor("xr", [OCT * I, NO * B], bf16, kind="ExternalInput")
    xbd_d = nc.dram_tensor("xbd", [GRP * I, NG, 128], bf16, kind="ExternalInput")
    ones4_d = nc.dram_tensor("ones4", [GRP * B, B], bf16, kind="ExternalInput")
    v_d = nc.dram_tensor("v", [B, DN], f32, kind="ExternalOutput")

    with tile.TileContext(nc) as tc:
        with (
            tc.tile_pool(name="persist", bufs=1) as pp,
            tc.tile_pool(name="wstream", bufs=4) as wp,
            tc.tile_pool(name="work", bufs=3) as wk,
            tc.tile_pool(name="small", bufs=2) as sm,
            tc.tile_pool(name="ups", bufs=2, space="PSUM") as ups_pool,
            tc.tile_pool(name="sps", bufs=1, space="PSUM") as sps_pool,
            tc.tile_pool(name="dram", bufs=1, space="DRAM") as dr,
        ):
            xr = pp.tile([OCT * I, NO * B], bf16)
            nc.sync.dma_start(xr[:], xr_d[:])
            xbd = pp.tile([GRP * I, NG, 128], bf16)
            nc.sync.dma_start(xbd[:], xbd_d[:])
            ones4 = pp.tile([GRP * B, B], bf16)
            nc.sync.dma_start(ones4[:], ones4_d[:])

            VrepC = pp.tile([128, DN], bf16)     # cumulative V, replicated
            Vacc = pp.tile([B, DN], f32)         # cumulative V = sum v_r
            zrow = pp.tile([1, 512], bf16)
            nc.vector.memset(zrow[:], 0.0)
            orow = pp.tile([1, 128], bf16)
            nc.vector.memset(orow[:], 0.0)
            eps_t = pp.tile([128, 1], f32)
            nc.vector.memset(eps_t[:], EPS)

            cc0_in = dr.tile([B, DN], f32)
            cc0_out = dr.tile([B, DN], f32)
            ccs_in = dr.tile([128, 512], f32)
            ccs_out = dr.tile([128, 512], f32)

            def squash_tail(s_sb, P, d_view, scl_bc, d_shape):
                """squash on s_sb [P, free]; returns v4 [P, free] f32.
                d_view: AP view [P, seg, d] with d strided for the reduce;
                scl_bc: fn scl -> broadcast AP matching s_sb's free shape."""
                sq = sm.tile(list(s_sb.shape), f32, name="sq", tag="it", bufs=2)
                nc.scalar.activation(sq[:], s_sb[:], AF.Square)
                ns2 = sm.tile([P, N // 4 if P == 128 else N], f32,
                              name="ns2", tag="pg", bufs=6)
                nc.vector.tensor_reduce(ns2[:], d_view(sq), axis=AX, op=ADD)
                onep = sm.tile(list(ns2.shape), f32, name="onep", tag="pg", bufs=6)
                nc.vector.tensor_scalar_add(onep[:], ns2[:], 1.0)
                rt = sm.tile(list(ns2.shape), f32, name="rt", tag="pg", bufs=6)
                nc.scalar.activation(rt[:], ns2[:], AF.Sqrt, bias=eps_t[:P])
                den = sm.tile(list(ns2.shape), f32, name="den", tag="pg", bufs=6)
                nc.vector.tensor_tensor(den[:], onep[:], rt[:], op=MUL)
                dinv = sm.tile(list(ns2.shape), f32, name="dinv", tag="pg", bufs=6)
                nc.vector.reciprocal(dinv[:], den[:])
                scl = sm.tile(list(ns2.shape), f32, name="scl", tag="pg", bufs=6)
                nc.vector.tensor_tensor(scl[:], ns2[:], dinv[:], op=MUL)
                v4 = sm.tile(list(s_sb.shape), f32, name="v4", tag="v4", bufs=2)
                bc = scl_bc(scl)
                nc.vector.tensor_tensor(d_shape(v4), d_shape(s_sb), bc, op=MUL)
                return v4

            def update_V(v4c, r):
                """Vacc (+)= v4c [32, DN]; VrepC = replicate(bf16(Vacc))."""
                if r == 0:
                    nc.vector.tensor_copy(Vacc[:], v4c[:])
                else:
                    nc.vector.tensor_add(Vacc[:], Vacc[:], v4c[:])
                vb = sm.tile([B, DN], bf16, name="vb", tag="it", bufs=2)
                nc.vector.tensor_copy(vb[:], Vacc[:])
                for k in range(4):
                    nc.sync.dma_start(VrepC[32 * k:32 * k + 32, :], vb[:])

            # ---------------- r0: x-stationary dense matmuls ----------------
            # start=True clears has_written for whole (rows x bank); clear
            # each bank once with a K=1 zero matmul, accumulate start=False.
            s0_ps = sps_pool.tile([B, DN], f32, name="s0ps", tag="sp")
            for q in range(4):
                nc.tensor.matmul(s0_ps[:, 512 * q:512 * q + 512],
                                 orow[:, :B], zrow[:],
                                 start=True, stop=False, skip_group_check=True)
            for o in range(NO):
                w2t = wp.tile([OCT * I, DN], bf16)
                nc.sync.dma_start(w2t[:], w2_d[o])
                for q in range(4):
                    nc.tensor.matmul(
                        s0_ps[:, 512 * q:512 * q + 512],
                        xr[:, B * o:B * o + B],
                        w2t[:, 512 * q:512 * q + 512],
                        start=False, stop=(o == NO - 1),
                        skip_group_check=True,
                    )
            s_ar = sm.tile([B, DN], f32, name="sar", tag="it", bufs=2)
            nc.scalar.activation(s_ar[:], s0_ps[:], AF.Copy, scale=1.0 / N)
            nc.sync.dma_start(cc0_in[:], s_ar[:])
            nc.gpsimd.collective_compute(
                "AllReduce", ADD, replica_groups=[list(range(NCORES))],
                ins=[cc0_in[:].opt()], outs=[cc0_out[:].opt()],
            )
            ssb0 = sm.tile([B, DN], f32, name="ssb0", tag="it", bufs=2)
            nc.sync.dma_start(ssb0[:], cc0_out[:])
            v4c = squash_tail(
                ssb0, B,
                lambda t: t[:].rearrange("b (q d nn) -> b q nn d",
                                         q=4, nn=16),
                lambda s: s[:].rearrange("b (q nn) -> b q nn", q=4)
                           [:, :, None, :].broadcast_to([B, 4, D, 16]),
                lambda t: t[:].rearrange("b (q d nn) -> b q d nn",
                                         q=4, nn=16))
            update_V(v4c, 0)

            # ---------------- r1, r2: routing sweeps ----------------
            for r in range(1, ROUTINGS):
                s_ps = sps_pool.tile([128, 512], f32, name="sps", tag="sp")
                pending_smm = []
                for g in range(NG):
                    w1t = wp.tile([GRP * I, DN], bf16)
                    nc.sync.dma_start(w1t[:], w1_d[g])

                    u_sb = wk.tile([128, 4, D, 16], bf16)   # [q, d, nn]
                    for h in range(2):
                        u_ps = ups_pool.tile([128, DN // 2], f32)
                        for k in range(2):
                            nc.tensor.matmul(
                                u_ps[:, 512 * k:512 * k + 512], xbd[:, g, :],
                                w1t[:, 1024 * h + 512 * k:
                                    1024 * h + 512 * k + 512],
                                start=True, stop=True,
                            )
                        if pending_smm:
                            pending_smm.pop(0)()
                        nc.scalar.activation(
                            u_sb[:, 2 * h:2 * h + 2]
                            .rearrange("p a b c -> p (a b c)"),
                            u_ps[:], AF.Copy)

                    tl = wk.tile([128, 4, D, 16], bf16, name="tl", tag="tl",
                                 bufs=3)
                    nc.vector.tensor_tensor(
                        tl[:].rearrange("p a b c -> p (a b c)"),
                        u_sb[:].rearrange("p a b c -> p (a b c)"),
                        VrepC[:], op=MUL)
                    # fold d 32 -> 1 with 5 halving TT-adds
                    with nc.allow_low_precision("bf16 routing logits"):
                        t = tl
                        w = D
                        while w > 1:
                            w //= 2
                            nt = sm.tile([128, 4, w, 16], bf16,
                                         name=f"fd{w}", tag=f"fd{w}", bufs=4)
                            nc.vector.tensor_tensor(
                                nt[:], t[:, :, 0:w, :], t[:, :, w:2 * w, :],
                                op=ADD)
                            t = nt
                    logits = t                    # [128, 4, 1, 16] = [128, 64]

                    e_t = sm.tile([128, N], bf16, name="et", tag="pg", bufs=6)
                    zsum = sm.tile([128, 1], f32, name="zsum", tag="pg", bufs=6)
                    nc.scalar.activation(
                        e_t[:], logits[:].rearrange("p a b c -> p (a b c)"),
                        AF.Exp, accum_out=zsum[:])
                    zrec = sm.tile([128, 1], f32, name="zrec", tag="pg", bufs=6)
                    nc.vector.reciprocal(zrec[:], zsum[:])
                    c_t = sm.tile([128, 4, 16], bf16, name="ct", tag="pg",
                                  bufs=6)   # [q, nn]
                    nc.vector.tensor_scalar_mul(
                        c_t[:].rearrange("p a b -> p (a b)"), e_t[:], zrec[:])

                    t2 = wk.tile([128, 4, D, 16], bf16, name="t2", tag="t2")
                    nc.vector.tensor_tensor(
                        t2[:, 0:2], u_sb[:, 0:2],
                        c_t[:, 0:2, None, :].broadcast_to([128, 2, D, 16]),
                        op=MUL)
                    nc.gpsimd.tensor_tensor(
                        t2[:, 2:4], u_sb[:, 2:4],
                        c_t[:, 2:4, None, :].broadcast_to([128, 2, D, 16]),
                        op=MUL)

                    def make_smm(t2=t2, g=g):
                        def emit():
                            for q in range(4):
                                nc.tensor.matmul(
                                    s_ps[32 * q:32 * q + 32, :],
                                    ones4[:],
                                    t2[:, q].rearrange("p a b -> p (a b)"),
                                    start=(g == 0), stop=(g == NG - 1),
                                    tile_position=(0, 32 * q),
                                    skip_group_check=True,
                                )
                        return emit
                    pending_smm.append(make_smm())
                while pending_smm:
                    pending_smm.pop(0)()

                # tail: AllReduce strips, squash in strips, compact v4
                s_ev = sm.tile([128, 512], f32, name="sev", tag="sev", bufs=2)
                nc.vector.tensor_copy(s_ev[:], s_ps[:])
                nc.sync.dma_start(ccs_in[:], s_ev[:])
                nc.gpsimd.collective_compute(
                    "AllReduce", ADD, replica_groups=[list(range(NCORES))],
                    ins=[ccs_in[:].opt()], outs=[ccs_out[:].opt()],
                )
                ssb_s = sm.tile([128, 512], f32, name="ssbs", tag="sev", bufs=2)
                nc.sync.dma_start(ssb_s[:], ccs_out[:])
                v4s = squash_tail(
                    ssb_s, 128,
                    lambda t: t[:].rearrange("p (d nn) -> p nn d", nn=16),
                    lambda s: s[:, None, :].broadcast_to([128, D, 16]),
                    lambda t: t[:].rearrange("p (d nn) -> p d nn", nn=16))
                v4c = sm.tile([B, DN], f32, name="v4c", tag="v4", bufs=2)
                for q in range(4):
                    nc.sync.dma_start(v4c[:, 512 * q:512 * q + 512],
                                      v4s[32 * q:32 * q + 32, :])
                if r < ROUTINGS - 1:
                    update_V(v4c, r)
                else:
                    nc.sync.dma_start(v_d[:], v4c[:])

    nc.compile()
    return nc


def prepare_in_maps(x: np.ndarray, W: np.ndarray):
    import ml_dtypes
    bf = ml_dtypes.bfloat16

    ones4 = np.tile(np.eye(B, dtype=np.float32), (GRP, 1)).astype(bf)

    in_maps = []
    for k in range(NCORES):
        jlo, jhi = k * JC, (k + 1) * JC
        Wc = W[:, jlo:jhi]                     # [N, 256, D, I]
        Wt = np.ascontiguousarray(Wc.transpose(1, 3, 2, 0))  # [j, i, d, n]
        # columns (q, d, nn): n = 16q + nn
        Wq = (Wt.reshape(JC, I, D, 4, 16).transpose(0, 1, 3, 2, 4)
              .reshape(JC, I, DN))
        w1 = Wq.reshape(NG, GRP * I, DN).astype(bf)
        w2 = Wq.reshape(NO, OCT * I, DN).astype(bf)
        xc = x[:, jlo:jhi]                     # [B, 256, I]
        xt_j = np.ascontiguousarray(xc.transpose(1, 2, 0))   # [j, i, b]
        xr = np.ascontiguousarray(
            xt_j.reshape(NO, OCT, I, B).transpose(1, 2, 0, 3)
        ).reshape(OCT * I, NO * B).astype(bf)
        xbd = np.zeros((GRP * I, NG, 128), dtype=np.float32)
        xv = xt_j.reshape(NG, GRP, I, B)       # [g, rr, i, b]
        for rr in range(GRP):
            xbd[16 * rr:16 * rr + 16, :, 32 * rr:32 * rr + 32] = \
                xv[:, rr].transpose(1, 0, 2)
        in_maps.append({
            "w1": np.ascontiguousarray(w1),
            "w2": np.ascontiguousarray(w2),
            "xr": xr,
            "xbd": xbd.astype(bf),
            "ones4": ones4,
        })
    return in_maps


def kernel(x: np.ndarray, W: np.ndarray) -> np.ndarray:
    from concourse.bass_utils import run_bass_kernel_spmd

    nc = _build()
    in_maps = prepare_in_maps(x, W)
    res = run_bass_kernel_spmd(nc, in_maps, list(range(NCORES)))
    v = np.asarray(res.results[0]["v"], dtype=np.float32)
    # v[b, q*512 + d*16 + nn] -> [B, N, D] with n = 16q + nn
    return np.ascontiguousarray(
        v.reshape(B, 4, D, 16).transpose(0, 1, 3, 2).reshape(B, N, D))


if __name__ == "__main__":
    rng = np.random.default_rng(0)
    x = rng.normal(size=(B, J, I)).astype(np.float32)
    W = rng.normal(size=(N, J, D, I)).astype(np.float32) * 0.05
    v = kernel(x, W)
    print(v.shape, v.dtype, np.abs(v).max())


# revision 20
# speedup vs baseline: 1.5489x; 1.2555x over previous
"""CapsuleLayer dynamic-routing kernel for 8 Trainium2 NeuronCores (v4).

Math (reference):
    u_hat[b,n,j,d] = sum_i W[n,j,d,i] * x[b,j,i]
    b = 0; for r in 0..2:
        c = softmax_n(b); s[b,n,d] = sum_j c*u_hat; v = squash_d(s)
        if r < 2: b += sum_d v*u_hat
    return v  [B, N, D]

Key identities:
  - logits_r = <V_r, u_hat> over d with V_r = v_0 + ... + v_{r-1}
    (logits accumulate additively, u_hat constant) -> no per-j state.
  - r0: c uniform = 1/N, so s0 = (1/N) sum_{j,i} x W — computed with
    x-stationary K=128 matmuls (full PE-row packing), accumulating
    [32b, 2048] directly in PSUM.  No transposes.

Sharding: J (2048) split 8 ways -> Jc=256/core; s AllReduce per
iteration (256 KiB); squash redundant on every core.

Free layout everywhere: f = q*512 + d*16 + nn  (n = 16q + nn).
This keeps every DVE op innermost-step-1 bf16 (2x mode), makes each
s-matmul quarter a contiguous 512-col slice, and each strip of the
s PSUM [32q+b, (d,nn)] compacts to [32, (q,d,nn)] with contiguous DMAs.

Per r>=1 group (4 j): K=64 block-diag-x matmuls -> u_ps; scalar-ACT
evac; tl = u*Vrep (DVE 2x); 5 halving TT-adds fold d -> logits;
exp+Z fused on Scalar (accum_out); c = e/Z; t2 = c*u split DVE/GpSimd;
4 col-tiled ones4 matmuls accumulate s strips in PSUM.
"""

import functools
import numpy as np

B, J, I = 32, 2048, 16
N, D = 64, 32
NCORES = 8
JC = J // NCORES          # 256 j per core
GRP = 4                   # j's per group
NG = JC // GRP            # 64 groups
OCT = 8                   # j's per r0 octet (K=128 stationary)
NO = JC // OCT            # 32 octets
DN = D * N                # 2048
ROUTINGS = 3
EPS = 1e-7


@functools.lru_cache(maxsize=1)
def _build():
    import concourse.mybir as mybir
    import concourse.bacc as bacc
    import concourse.tile as tile

    f32 = mybir.dt.float32
    bf16 = mybir.dt.bfloat16
    MUL = mybir.AluOpType.mult
    ADD = mybir.AluOpType.add
    AF = mybir.ActivationFunctionType
    AX = mybir.AxisListType.X

    nc = bacc.Bacc("TRN2", target_bir_lowering=False, debug=False,
                   num_devices=NCORES)

    w1_d = nc.dram_tensor("w1", [NG, GRP * I, DN], bf16, kind="ExternalInput")
    w2_d = nc.dram_tensor("w2", [NO, OCT * I, DN], bf16, kind="ExternalInput")
    xr_d = nc.dram_tensor("xr", [OCT * I, NO * B], bf16, kind="ExternalInput")
    xbd_d = nc.dram_tensor("xbd", [GRP * I, NG, 128], bf16, kind="ExternalInput")
    ones4_d = nc.dram_tensor("ones4", [GRP * B, B], bf16, kind="ExternalInput")
    v_d = nc.dram_tensor("v", [B, DN], f32, kind="ExternalOutput")

    with tile.TileContext(nc) as tc:
        with (
            tc.tile_pool(name="persist", bufs=1) as pp,
            tc.tile_pool(name="wstream", bufs=4) as wp,
            tc.tile_pool(name="work", bufs=3) as wk,
            tc.tile_pool(name="small", bufs=2) as sm,
            tc.tile_pool(name="ups", bufs=2, space="PSUM") as ups_pool,
            tc.tile_pool(name="sps", bufs=1, space="PSUM") as sps_pool,
            tc.tile_pool(name="dram", bufs=1, space="DRAM") as dr,
        ):
            xr = pp.tile([OCT * I, NO * B], bf16)
            nc.sync.dma_start(xr[:], xr_d[:])
            xbd = pp.tile([GRP * I, NG, 128], bf16)
            nc.sync.dma_start(xbd[:], xbd_d[:])
            ones4 = pp.tile([GRP * B, B], bf16)
            nc.sync.dma_start(ones4[:], ones4_d[:])

            VrepC = pp.tile([128, DN], bf16)     # cumulative V, replicated
            Vacc = pp.tile([128, 512], f32)      # cumulative V, strip layout
            zrow = pp.tile([1, 512], bf16)
            nc.vector.memset(zrow[:], 0.0)
            orow = pp.tile([1, 128], bf16)
            nc.vector.memset(orow[:], 0.0)
            eps_t = pp.tile([128, 1], f32)
            nc.vector.memset(eps_t[:], EPS)

            cc0_in = dr.tile([B, DN], f32)
            cc0_out = dr.tile([B, DN], f32)
            ccs_in = dr.tile([128, 512], bf16)
            ccs_out = dr.tile([128, 512], bf16)

            def squash_tail(s_sb, P, d_view, scl_bc, d_shape):
                """squash on s_sb [P, free]; returns v4 [P, free] f32.
                d_view: AP view [P, seg, d] with d strided for the reduce;
                scl_bc: fn scl -> broadcast AP matching s_sb's free shape."""
                sq = sm.tile(list(s_sb.shape), f32, name="sq", tag="it", bufs=2)
                nc.scalar.activation(sq[:], s_sb[:], AF.Square)
                ns2 = sm.tile([P, N // 4 if P == 128 else N], f32,
                              name="ns2", tag="pg", bufs=6)
                nc.vector.tensor_reduce(ns2[:], d_view(sq), axis=AX, op=ADD)
                onep = sm.tile(list(ns2.shape), f32, name="onep", tag="pg", bufs=6)
                nc.vector.tensor_scalar_add(onep[:], ns2[:], 1.0)
                rt = sm.tile(list(ns2.shape), f32, name="rt", tag="pg", bufs=6)
                nc.scalar.activation(rt[:], ns2[:], AF.Sqrt, bias=eps_t[:P])
                den = sm.tile(list(ns2.shape), f32, name="den", tag="pg", bufs=6)
                nc.vector.tensor_tensor(den[:], onep[:], rt[:], op=MUL)
                dinv = sm.tile(list(ns2.shape), f32, name="dinv", tag="pg", bufs=6)
                nc.vector.reciprocal(dinv[:], den[:])
                scl = sm.tile(list(ns2.shape), f32, name="scl", tag="pg", bufs=6)
                nc.vector.tensor_tensor(scl[:], ns2[:], dinv[:], op=MUL)
                v4 = sm.tile(list(s_sb.shape), f32, name="v4", tag="v4", bufs=2)
                bc = scl_bc(scl)
                nc.vector.tensor_tensor(d_shape(v4), d_shape(s_sb), bc, op=MUL)
                return v4

            def update_V(v4s, r):
                """Vacc (+)= v4s [128,512] strips; VrepC = replicate."""
                if r == 0:
                    nc.vector.tensor_copy(Vacc[:], v4s[:])
                else:
                    nc.vector.tensor_add(Vacc[:], Vacc[:], v4s[:])
                vb = sm.tile([128, 512], bf16, name="vb", tag="it", bufs=2)
                nc.vector.tensor_copy(vb[:], Vacc[:])
                for k in range(4):
                    for q in range(4):
                        nc.sync.dma_start(
                            VrepC[32 * k:32 * k + 32, 512 * q:512 * q + 512],
                            vb[32 * q:32 * q + 32, :])

            # ---------------- r0: x-stationary dense matmuls ----------------
            # start=True clears has_written for whole (rows x bank); clear
            # each bank once with a K=1 zero matmul, accumulate start=False.
            s0_ps = sps_pool.tile([B, DN], f32, name="s0ps", tag="sp")
            for q in range(4):
                nc.tensor.matmul(s0_ps[:, 512 * q:512 * q + 512],
                                 orow[:, :B], zrow[:],
                                 start=True, stop=False, skip_group_check=True)
            for o in range(NO):
                w2t = wp.tile([OCT * I, DN], bf16)
                nc.sync.dma_start(w2t[:], w2_d[o])
                for q in range(4):
                    nc.tensor.matmul(
                        s0_ps[:, 512 * q:512 * q + 512],
                        xr[:, B * o:B * o + B],
                        w2t[:, 512 * q:512 * q + 512],
                        start=False, stop=(o == NO - 1),
                        skip_group_check=True,
                    )
            s_ar = sm.tile([B, DN], f32, name="sar", tag="it", bufs=2)
            nc.scalar.activation(s_ar[:], s0_ps[:], AF.Copy, scale=1.0 / N)
            nc.sync.dma_start(cc0_in[:], s_ar[:])
            nc.gpsimd.collective_compute(
                "AllReduce", ADD, replica_groups=[list(range(NCORES))],
                ins=[cc0_in[:].opt()], outs=[cc0_out[:].opt()],
            )
            ssb0 = sm.tile([B, DN], f32, name="ssb0", tag="it", bufs=2)
            nc.sync.dma_start(ssb0[:], cc0_out[:])
            v4c = squash_tail(
                ssb0, B,
                lambda t: t[:].rearrange("b (q d nn) -> b q nn d",
                                         q=4, nn=16),
                lambda s: s[:].rearrange("b (q nn) -> b q nn", q=4)
                           [:, :, None, :].broadcast_to([B, 4, D, 16]),
                lambda t: t[:].rearrange("b (q d nn) -> b q d nn",
                                         q=4, nn=16))
            v4s0 = sm.tile([128, 512], f32, name="v4s0", tag="v4", bufs=2)
            for q in range(4):
                nc.sync.dma_start(v4s0[32 * q:32 * q + 32, :],
                                  v4c[:, 512 * q:512 * q + 512])
            update_V(v4s0, 0)

            # ---------------- r1, r2: routing sweeps ----------------
            for r in range(1, ROUTINGS):
                s_ps = sps_pool.tile([128, 512], f32, name="sps", tag="sp")
                pending_smm = []
                for g in range(NG):
                    w1t = wp.tile([GRP * I, DN], bf16)
                    nc.sync.dma_start(w1t[:], w1_d[g])

                    u_sb = wk.tile([128, 4, D, 16], bf16)   # [q, d, nn]
                    for h in range(2):
                        u_ps = ups_pool.tile([128, DN // 2], f32)
                        for k in range(2):
                            nc.tensor.matmul(
                                u_ps[:, 512 * k:512 * k + 512], xbd[:, g, :],
                                w1t[:, 1024 * h + 512 * k:
                                    1024 * h + 512 * k + 512],
                                start=True, stop=True,
                            )
                        if pending_smm:
                            pending_smm.pop(0)()
                        nc.scalar.activation(
                            u_sb[:, 2 * h:2 * h + 2]
                            .rearrange("p a b c -> p (a b c)"),
                            u_ps[:], AF.Copy)

                    tl = wk.tile([128, 4, D, 16], bf16, name="tl", tag="tl",
                                 bufs=3)
                    nc.vector.tensor_tensor(
                        tl[:].rearrange("p a b c -> p (a b c)"),
                        u_sb[:].rearrange("p a b c -> p (a b c)"),
                        VrepC[:], op=MUL)
                    # fold d 32 -> 1 with 5 halving TT-adds
                    with nc.allow_low_precision("bf16 routing logits"):
                        t = tl
                        w = D
                        while w > 1:
                            w //= 2
                            nt = sm.tile([128, 4, w, 16], bf16,
                                         name=f"fd{w}", tag=f"fd{w}", bufs=4)
                            nc.vector.tensor_tensor(
                                nt[:], t[:, :, 0:w, :], t[:, :, w:2 * w, :],
                                op=ADD)
                            t = nt
                    logits = t                    # [128, 4, 1, 16] = [128, 64]

                    e_t = sm.tile([128, N], bf16, name="et", tag="pg", bufs=6)
                    zsum = sm.tile([128, 1], f32, name="zsum", tag="pg", bufs=6)
                    nc.scalar.activation(
                        e_t[:], logits[:].rearrange("p a b c -> p (a b c)"),
                        AF.Exp, accum_out=zsum[:])
                    zrec = sm.tile([128, 1], f32, name="zrec", tag="pg", bufs=6)
                    nc.vector.reciprocal(zrec[:], zsum[:])
                    c_t = sm.tile([128, 4, 16], bf16, name="ct", tag="pg",
                                  bufs=6)   # [q, nn]
                    nc.vector.tensor_scalar_mul(
                        c_t[:].rearrange("p a b -> p (a b)"), e_t[:], zrec[:])

                    t2 = wk.tile([128, 4, D, 16], bf16, name="t2", tag="t2")
                    nc.vector.tensor_tensor(
                        t2[:], u_sb[:],
                        c_t[:, :, None, :].broadcast_to([128, 4, D, 16]),
                        op=MUL)

                    def make_smm(t2=t2, g=g):
                        def emit():
                            for q in range(4):
                                nc.tensor.matmul(
                                    s_ps[32 * q:32 * q + 32, :],
                                    ones4[:],
                                    t2[:, q].rearrange("p a b -> p (a b)"),
                                    start=(g == 0), stop=(g == NG - 1),
                                    tile_position=(0, 32 * q),
                                    skip_group_check=True,
                                )
                        return emit
                    pending_smm.append(make_smm())
                while pending_smm:
                    pending_smm.pop(0)()

                # tail: AllReduce strips, squash in strips, compact v4
                s_ev = sm.tile([128, 512], bf16, name="sev", tag="sev", bufs=2)
                nc.vector.tensor_copy(s_ev[:], s_ps[:])
                nc.sync.dma_start(ccs_in[:], s_ev[:])
                nc.gpsimd.collective_compute(
                    "AllReduce", ADD, replica_groups=[list(range(NCORES))],
                    ins=[ccs_in[:].opt()], outs=[ccs_out[:].opt()],
                )
                ssb_s = sm.tile([128, 512], bf16, name="ssbs", tag="sev", bufs=2)
                nc.sync.dma_start(ssb_s[:], ccs_out[:])
                v4s = squash_tail(
                    ssb_s, 128,
                    lambda t: t[:].rearrange("p (d nn) -> p nn d", nn=16),
                    lambda s: s[:, None, :].broadcast_to([128, D, 16]),
                    lambda t: t[:].rearrange("p (d nn) -> p d nn", nn=16))
                if r < ROUTINGS - 1:
                    update_V(v4s, r)
                else:
                    v4c = sm.tile([B, DN], f32, name="v4c", tag="v4", bufs=2)
                    for q in range(4):
                        nc.sync.dma_start(v4c[:, 512 * q:512 * q + 512],
                                          v4s[32 * q:32 * q + 32, :])
                    nc.sync.dma_start(v_d[:], v4c[:])

    nc.compile()
    return nc


def prepare_in_maps(x: np.ndarray, W: np.ndarray):
    import ml_dtypes
    bf = ml_dtypes.bfloat16

    ones4 = np.tile(np.eye(B, dtype=np.float32), (GRP, 1)).astype(bf)

    in_maps = []
    for k in range(NCORES):
        jlo, jhi = k * JC, (k + 1) * JC
        Wc = W[:, jlo:jhi]                     # [N, 256, D, I]
        Wt = np.ascontiguousarray(Wc.transpose(1, 3, 2, 0))  # [j, i, d, n]
        # columns (q, d, nn): n = 16q + nn
        Wq = (Wt.reshape(JC, I, D, 4, 16).transpose(0, 1, 3, 2, 4)
              .reshape(JC, I, DN))
        w1 = Wq.reshape(NG, GRP * I, DN).astype(bf)
        w2 = Wq.reshape(NO, OCT * I, DN).astype(bf)
        xc = x[:, jlo:jhi]                     # [B, 256, I]
        xt_j = np.ascontiguousarray(xc.transpose(1, 2, 0))   # [j, i, b]
        xr = np.ascontiguousarray(
            xt_j.reshape(NO, OCT, I, B).transpose(1, 2, 0, 3)
        ).reshape(OCT * I, NO * B).astype(bf)
        xbd = np.zeros((GRP * I, NG, 128), dtype=np.float32)
        xv = xt_j.reshape(NG, GRP, I, B)       # [g, rr, i, b]
        for rr in range(GRP):
            xbd[16 * rr:16 * rr + 16, :, 32 * rr:32 * rr + 32] = \
                xv[:, rr].transpose(1, 0, 2)
        in_maps.append({
            "w1": np.ascontiguousarray(w1),
            "w2": np.ascontiguousarray(w2),
            "xr": xr,
            "xbd": xbd.astype(bf),
            "ones4": ones4,
        })
    return in_maps


def kernel(x: np.ndarray, W: np.ndarray) -> np.ndarray:
    from concourse.bass_utils import run_bass_kernel_spmd

    nc = _build()
    in_maps = prepare_in_maps(x, W)
    res = run_bass_kernel_spmd(nc, in_maps, list(range(NCORES)))
    v = np.asarray(res.results[0]["v"], dtype=np.float32)
    # v[b, q*512 + d*16 + nn] -> [B, N, D] with n = 16q + nn
    return np.ascontiguousarray(
        v.reshape(B, 4, D, 16).transpose(0, 1, 3, 2).reshape(B, N, D))


if __name__ == "__main__":
    rng = np.random.default_rng(0)
    x = rng.normal(size=(B, J, I)).astype(np.float32)
    W = rng.normal(size=(N, J, D, I)).astype(np.float32) * 0.05
    v = kernel(x, W)
    print(v.shape, v.dtype, np.abs(v).max())


# revision 21
# speedup vs baseline: 1.6701x; 1.0782x over previous
"""CapsuleLayer dynamic-routing kernel for 8 Trainium2 NeuronCores (v4).

Math (reference):
    u_hat[b,n,j,d] = sum_i W[n,j,d,i] * x[b,j,i]
    b = 0; for r in 0..2:
        c = softmax_n(b); s[b,n,d] = sum_j c*u_hat; v = squash_d(s)
        if r < 2: b += sum_d v*u_hat
    return v  [B, N, D]

Key identities:
  - logits_r = <V_r, u_hat> over d with V_r = v_0 + ... + v_{r-1}
    (logits accumulate additively, u_hat constant) -> no per-j state.
  - r0: c uniform = 1/N, so s0 = (1/N) sum_{j,i} x W — computed with
    x-stationary K=128 matmuls (full PE-row packing), accumulating
    [32b, 2048] directly in PSUM.  No transposes.

Sharding: J (2048) split 8 ways -> Jc=256/core; s AllReduce per
iteration (256 KiB); squash redundant on every core.

Free layout everywhere: f = q*512 + d*16 + nn  (n = 16q + nn).
This keeps every DVE op innermost-step-1 bf16 (2x mode), makes each
s-matmul quarter a contiguous 512-col slice, and each strip of the
s PSUM [32q+b, (d,nn)] compacts to [32, (q,d,nn)] with contiguous DMAs.

Per r>=1 group (4 j): K=64 block-diag-x matmuls -> u_ps; scalar-ACT
evac; tl = u*Vrep (DVE 2x); 5 halving TT-adds fold d -> logits;
exp+Z fused on Scalar (accum_out); c = e/Z; t2 = c*u split DVE/GpSimd;
4 col-tiled ones4 matmuls accumulate s strips in PSUM.
"""

import functools
import numpy as np

B, J, I = 32, 2048, 16
N, D = 64, 32
NCORES = 8
JC = J // NCORES          # 256 j per core
GRP = 4                   # j's per group
NG = JC // GRP            # 64 groups
OCT = 8                   # j's per r0 octet (K=128 stationary)
NO = JC // OCT            # 32 octets
DN = D * N                # 2048
ROUTINGS = 3
EPS = 1e-7


@functools.lru_cache(maxsize=1)
def _build():
    import concourse.mybir as mybir
    import concourse.bacc as bacc
    import concourse.tile as tile

    f32 = mybir.dt.float32
    bf16 = mybir.dt.bfloat16
    MUL = mybir.AluOpType.mult
    ADD = mybir.AluOpType.add
    AF = mybir.ActivationFunctionType
    AX = mybir.AxisListType.X

    nc = bacc.Bacc("TRN2", target_bir_lowering=False, debug=False,
                   num_devices=NCORES)

    w1_d = nc.dram_tensor("w1", [NG, GRP * I, DN], bf16, kind="ExternalInput")
    w2_d = nc.dram_tensor("w2", [NO, OCT * I, DN], bf16, kind="ExternalInput")
    xr_d = nc.dram_tensor("xr", [OCT * I, NO * B], bf16, kind="ExternalInput")
    xbd_d = nc.dram_tensor("xbd", [GRP * I, NG, 128], bf16, kind="ExternalInput")
    ones4_d = nc.dram_tensor("ones4", [GRP * B, B], bf16, kind="ExternalInput")
    v_d = nc.dram_tensor("v", [B, DN], f32, kind="ExternalOutput")

    with tile.TileContext(nc) as tc:
        with (
            tc.tile_pool(name="persist", bufs=1) as pp,
            tc.tile_pool(name="wstream", bufs=4) as wp,
            tc.tile_pool(name="work", bufs=4) as wk,
            tc.tile_pool(name="small", bufs=2) as sm,
            tc.tile_pool(name="ups", bufs=3, space="PSUM") as ups_pool,
            tc.tile_pool(name="sps", bufs=1, space="PSUM") as sps_pool,
            tc.tile_pool(name="dram", bufs=1, space="DRAM") as dr,
        ):
            xr = pp.tile([OCT * I, NO * B], bf16)
            nc.sync.dma_start(xr[:], xr_d[:])
            xbd = pp.tile([GRP * I, NG, 128], bf16)
            nc.sync.dma_start(xbd[:], xbd_d[:])
            ones4 = pp.tile([GRP * B, B], bf16)
            nc.sync.dma_start(ones4[:], ones4_d[:])

            VrepC = pp.tile([128, DN], bf16)     # cumulative V, replicated
            Vacc = pp.tile([128, 512], f32)      # cumulative V, strip layout
            zrow = pp.tile([1, 512], bf16)
            nc.vector.memset(zrow[:], 0.0)
            orow = pp.tile([1, 128], bf16)
            nc.vector.memset(orow[:], 0.0)
            eps_t = pp.tile([128, 1], f32)
            nc.vector.memset(eps_t[:], EPS)

            cc0_in = dr.tile([B, DN], bf16)
            cc0_out = dr.tile([B, DN], bf16)
            ccs_in = dr.tile([128, 512], bf16)
            ccs_out = dr.tile([128, 512], bf16)

            def squash_tail(s_sb, P, d_view, scl_bc, d_shape):
                """squash on s_sb [P, free]; returns v4 [P, free] f32.
                d_view: AP view [P, seg, d] with d strided for the reduce;
                scl_bc: fn scl -> broadcast AP matching s_sb's free shape."""
                sq = sm.tile(list(s_sb.shape), f32, name="sq", tag="it", bufs=2)
                nc.scalar.activation(sq[:], s_sb[:], AF.Square)
                ns2 = sm.tile([P, N // 4 if P == 128 else N], f32,
                              name="ns2", tag="pg", bufs=6)
                nc.vector.tensor_reduce(ns2[:], d_view(sq), axis=AX, op=ADD)
                onep = sm.tile(list(ns2.shape), f32, name="onep", tag="pg", bufs=6)
                nc.vector.tensor_scalar_add(onep[:], ns2[:], 1.0)
                rt = sm.tile(list(ns2.shape), f32, name="rt", tag="pg", bufs=6)
                nc.scalar.activation(rt[:], ns2[:], AF.Sqrt, bias=eps_t[:P])
                den = sm.tile(list(ns2.shape), f32, name="den", tag="pg", bufs=6)
                nc.vector.tensor_tensor(den[:], onep[:], rt[:], op=MUL)
                dinv = sm.tile(list(ns2.shape), f32, name="dinv", tag="pg", bufs=6)
                nc.vector.reciprocal(dinv[:], den[:])
                scl = sm.tile(list(ns2.shape), f32, name="scl", tag="pg", bufs=6)
                nc.vector.tensor_tensor(scl[:], ns2[:], dinv[:], op=MUL)
                v4 = sm.tile(list(s_sb.shape), f32, name="v4", tag="v4", bufs=2)
                bc = scl_bc(scl)
                nc.vector.tensor_tensor(d_shape(v4), d_shape(s_sb), bc, op=MUL)
                return v4

            def update_V(v4s, r):
                """Vacc (+)= v4s [128,512] strips; VrepC = replicate."""
                if r == 0:
                    nc.vector.tensor_copy(Vacc[:], v4s[:])
                else:
                    nc.vector.tensor_add(Vacc[:], Vacc[:], v4s[:])
                vb = sm.tile([128, 512], bf16, name="vb", tag="it", bufs=2)
                nc.vector.tensor_copy(vb[:], Vacc[:])
                for k in range(4):
                    for q in range(4):
                        nc.sync.dma_start(
                            VrepC[32 * k:32 * k + 32, 512 * q:512 * q + 512],
                            vb[32 * q:32 * q + 32, :])

            # ---------------- r0: x-stationary dense matmuls ----------------
            # start=True clears has_written for whole (rows x bank); clear
            # each bank once with a K=1 zero matmul, accumulate start=False.
            s0a = ups_pool.tile([B, DN // 2], f32, name="s0a", tag="u_ps")
            s0b = ups_pool.tile([B, DN // 2], f32, name="s0b", tag="u_ps")
            s0t = [s0a, s0a, s0b, s0b]
            for q in range(4):
                nc.tensor.matmul(s0t[q][:, 512 * (q % 2):512 * (q % 2) + 512],
                                 orow[:, :B], zrow[:],
                                 start=True, stop=False, skip_group_check=True)
            for o in range(NO):
                w2t = wp.tile([OCT * I, DN], bf16)
                nc.sync.dma_start(w2t[:], w2_d[o])
                for q in range(4):
                    nc.tensor.matmul(
                        s0t[q][:, 512 * (q % 2):512 * (q % 2) + 512],
                        xr[:, B * o:B * o + B],
                        w2t[:, 512 * q:512 * q + 512],
                        start=False, stop=(o == NO - 1),
                        skip_group_check=True,
                    )
            s_ar = sm.tile([B, DN], bf16, name="sar", tag="it", bufs=2)
            nc.scalar.activation(s_ar[:, 0:1024], s0a[:], AF.Copy,
                                 scale=1.0 / N)
            nc.scalar.activation(s_ar[:, 1024:2048], s0b[:], AF.Copy,
                                 scale=1.0 / N)
            nc.sync.dma_start(cc0_in[:], s_ar[:])
            nc.gpsimd.collective_compute(
                "AllReduce", ADD, replica_groups=[list(range(NCORES))],
                ins=[cc0_in[:].opt()], outs=[cc0_out[:].opt()],
            )
            ssb0 = sm.tile([B, DN], bf16, name="ssb0", tag="it", bufs=2)
            nc.sync.dma_start(ssb0[:], cc0_out[:])
            v4c = squash_tail(
                ssb0, B,
                lambda t: t[:].rearrange("b (q d nn) -> b q nn d",
                                         q=4, nn=16),
                lambda s: s[:].rearrange("b (q nn) -> b q nn", q=4)
                           [:, :, None, :].broadcast_to([B, 4, D, 16]),
                lambda t: t[:].rearrange("b (q d nn) -> b q d nn",
                                         q=4, nn=16))
            v4s0 = sm.tile([128, 512], f32, name="v4s0", tag="v4", bufs=2)
            for q in range(4):
                nc.sync.dma_start(v4s0[32 * q:32 * q + 32, :],
                                  v4c[:, 512 * q:512 * q + 512])
            update_V(v4s0, 0)

            # ---------------- r1, r2: routing sweeps ----------------
            for r in range(1, ROUTINGS):
                s_ps = sps_pool.tile([128, 512], f32, name="sps", tag="sp")
                pending_smm = []
                for g in range(NG):
                    w1t = wp.tile([GRP * I, DN], bf16)
                    nc.sync.dma_start(w1t[:], w1_d[g])

                    u_sb = wk.tile([128, 4, D, 16], bf16)   # [q, d, nn]
                    for h in range(2):
                        u_ps = ups_pool.tile([128, DN // 2], f32, name="u_ps", tag="u_ps")
                        for k in range(2):
                            nc.tensor.matmul(
                                u_ps[:, 512 * k:512 * k + 512], xbd[:, g, :],
                                w1t[:, 1024 * h + 512 * k:
                                    1024 * h + 512 * k + 512],
                                start=True, stop=True,
                            )
                        if pending_smm:
                            pending_smm.pop(0)()
                        nc.scalar.activation(
                            u_sb[:, 2 * h:2 * h + 2]
                            .rearrange("p a b c -> p (a b c)"),
                            u_ps[:], AF.Copy)

                    tl = wk.tile([128, 4, D, 16], bf16, name="tl", tag="tl",
                                 bufs=3)
                    nc.vector.tensor_tensor(
                        tl[:].rearrange("p a b c -> p (a b c)"),
                        u_sb[:].rearrange("p a b c -> p (a b c)"),
                        VrepC[:], op=MUL)
                    # fold d 32 -> 1 with 5 halving TT-adds
                    with nc.allow_low_precision("bf16 routing logits"):
                        t = tl
                        w = D
                        while w > 1:
                            w //= 2
                            nt = sm.tile([128, 4, w, 16], bf16,
                                         name=f"fd{w}", tag=f"fd{w}", bufs=4)
                            nc.vector.tensor_tensor(
                                nt[:], t[:, :, 0:w, :], t[:, :, w:2 * w, :],
                                op=ADD)
                            t = nt
                    logits = t                    # [128, 4, 1, 16] = [128, 64]

                    e_t = sm.tile([128, N], bf16, name="et", tag="pg", bufs=6)
                    zsum = sm.tile([128, 1], f32, name="zsum", tag="pg", bufs=6)
                    nc.scalar.activation(
                        e_t[:], logits[:].rearrange("p a b c -> p (a b c)"),
                        AF.Exp, accum_out=zsum[:])
                    zrec = sm.tile([128, 1], f32, name="zrec", tag="pg", bufs=6)
                    nc.vector.reciprocal(zrec[:], zsum[:])
                    c_t = sm.tile([128, 4, 16], bf16, name="ct", tag="pg",
                                  bufs=6)   # [q, nn]
                    nc.vector.tensor_scalar_mul(
                        c_t[:].rearrange("p a b -> p (a b)"), e_t[:], zrec[:])

                    t2 = wk.tile([128, 4, D, 16], bf16, name="t2", tag="t2")
                    nc.vector.tensor_tensor(
                        t2[:], u_sb[:],
                        c_t[:, :, None, :].broadcast_to([128, 4, D, 16]),
                        op=MUL)

                    def make_smm(t2=t2, g=g):
                        def emit():
                            for q in range(4):
                                nc.tensor.matmul(
                                    s_ps[32 * q:32 * q + 32, :],
                                    ones4[:],
                                    t2[:, q].rearrange("p a b -> p (a b)"),
                                    start=(g == 0), stop=(g == NG - 1),
                                    tile_position=(0, 32 * q),
                                    skip_group_check=True,
                                )
                        return emit
                    pending_smm.append(make_smm())
                while pending_smm:
                    pending_smm.pop(0)()

                # tail: AllReduce strips, squash in strips, compact v4
                s_ev = sm.tile([128, 512], bf16, name="sev", tag="sev", bufs=2)
                nc.vector.tensor_copy(s_ev[:], s_ps[:])
                nc.sync.dma_start(ccs_in[:], s_ev[:])
                nc.gpsimd.collective_compute(
                    "AllReduce", ADD, replica_groups=[list(range(NCORES))],
                    ins=[ccs_in[:].opt()], outs=[ccs_out[:].opt()],
                )
                ssb_s = sm.tile([128, 512], bf16, name="ssbs", tag="sev", bufs=2)
                nc.sync.dma_start(ssb_s[:], ccs_out[:])
                v4s = squash_tail(
                    ssb_s, 128,
                    lambda t: t[:].rearrange("p (d nn) -> p nn d", nn=16),
                    lambda s: s[:, None, :].broadcast_to([128, D, 16]),
                    lambda t: t[:].rearrange("p (d nn) -> p d nn", nn=16))
                if r < ROUTINGS - 1:
                    update_V(v4s, r)
                else:
                    v4c = sm.tile([B, DN], f32, name="v4c", tag="v4", bufs=2)
                    for q in range(4):
                        nc.sync.dma_start(v4c[:, 512 * q:512 * q + 512],
                                          v4s[32 * q:32 * q + 32, :])
                    nc.sync.dma_start(v_d[:], v4c[:])

    nc.compile()
    return nc


def prepare_in_maps(x: np.ndarray, W: np.ndarray):
    import ml_dtypes
    bf = ml_dtypes.bfloat16

    ones4 = np.tile(np.eye(B, dtype=np.float32), (GRP, 1)).astype(bf)

    in_maps = []
    for k in range(NCORES):
        jlo, jhi = k * JC, (k + 1) * JC
        Wc = W[:, jlo:jhi]                     # [N, 256, D, I]
        Wt = np.ascontiguousarray(Wc.transpose(1, 3, 2, 0))  # [j, i, d, n]
        # columns (q, d, nn): n = 16q + nn
        Wq = (Wt.reshape(JC, I, D, 4, 16).transpose(0, 1, 3, 2, 4)
              .reshape(JC, I, DN))
        w1 = Wq.reshape(NG, GRP * I, DN).astype(bf)
        w2 = Wq.reshape(NO, OCT * I, DN).astype(bf)
        xc = x[:, jlo:jhi]                     # [B, 256, I]
        xt_j = np.ascontiguousarray(xc.transpose(1, 2, 0))   # [j, i, b]
        xr = np.ascontiguousarray(
            xt_j.reshape(NO, OCT, I, B).transpose(1, 2, 0, 3)
        ).reshape(OCT * I, NO * B).astype(bf)
        xbd = np.zeros((GRP * I, NG, 128), dtype=np.float32)
        xv = xt_j.reshape(NG, GRP, I, B)       # [g, rr, i, b]
        for rr in range(GRP):
            xbd[16 * rr:16 * rr + 16, :, 32 * rr:32 * rr + 32] = \
                xv[:, rr].transpose(1, 0, 2)
        in_maps.append({
            "w1": np.ascontiguousarray(w1),
            "w2": np.ascontiguousarray(w2),
            "xr": xr,
            "xbd": xbd.astype(bf),
            "ones4": ones4,
        })
    return in_maps


def kernel(x: np.ndarray, W: np.ndarray) -> np.ndarray:
    from concourse.bass_utils import run_bass_kernel_spmd

    nc = _build()
    in_maps = prepare_in_maps(x, W)
    res = run_bass_kernel_spmd(nc, in_maps, list(range(NCORES)))
    v = np.asarray(res.results[0]["v"], dtype=np.float32)
    # v[b, q*512 + d*16 + nn] -> [B, N, D] with n = 16q + nn
    return np.ascontiguousarray(
        v.reshape(B, 4, D, 16).transpose(0, 1, 3, 2).reshape(B, N, D))


if __name__ == "__main__":
    rng = np.random.default_rng(0)
    x = rng.normal(size=(B, J, I)).astype(np.float32)
    W = rng.normal(size=(N, J, D, I)).astype(np.float32) * 0.05
    v = kernel(x, W)
    print(v.shape, v.dtype, np.abs(v).max())
